# revision 49
# baseline (speedup 1.0000x reference)
"""Single-head causal attention (B=4, T=2048, D=1024, HS=64) on 8 TRN2 cores.

Sharding: 2 cores per batch element. Query blocks (128 rows, 16/batch) are
fold-split for causal balance:
  role 0 (cores 0-3): blocks {0,1,2,3,12,13,14,15} of batch (core_id % 4)
  role 1 (cores 4-7): blocks {4..11}              of batch (core_id % 4)

Precision scheme (fp16, 10 mantissa bits):
  host: x.T in fp16 single plane (4MB/batch DMA), W pre-transposed fp16
  hi/lo pairs. k,v,q projections: 2 matmul groups (x*wh + x*wl) in fp32
  PSUM — error dominated by the single fp16 rounding of x (~2^-11 rel).
  scores: k,q re-split into fp16 hi/lo pairs on device;
    S = [qh;qh]^T.[kl;kh] + [0;ql]^T.[kl;kh]  (2 matmuls per 512-chunk)
  softmax: chunked row-max (DVE) + exp on ACT (scale=8, bias=-8*max),
  E fp16; E^T via PE transposes batched 4-per-PSUM-tile; AV inverted:
  out[q,h] accumulates lhsT=E^T-block (stationary, FWL) x rhs=v-natural
  so the result lands layout-natural and 1/Z applies per-partition.
"""

import numpy as np

N_CORES = 8
B, T, D, HS = 4, 2048, 1024, 64
P = 128
NT = T // P        # 16
ND = D // P        # 8
NCH = 4            # 512-wide t chunks
SCALE = 8.0        # sqrt(HS)
NEG = -1.0e30

ROLE_BLOCKS = [
    [0, 1, 2, 3, 12, 13, 14, 15],
    [4, 5, 6, 7, 8, 9, 10, 11],
]
ROLE_QCHUNKS = [[0, 3], [1, 2]]  # 512-chunk indices holding each role's q rows


def _block_qloc(role, j):
    if role == 0:
        return (0, 128 * j) if j < 4 else (1, 128 * (j - 12))
    return (0, 128 * (j - 4)) if j < 8 else (1, 128 * (j - 8))


_COMPILED = None


def _build():
    import concourse.bass as bass
    import concourse.tile as tile
    from concourse import bacc, mybir

    f32 = mybir.dt.float32
    f16 = mybir.dt.float16
    EXP = mybir.ActivationFunctionType.Exp
    AX = mybir.AxisListType.X

    nc = bacc.Bacc("TRN2", target_bir_lowering=False, debug=False,
                   num_devices=N_CORES)

    # x^T per batch, fp16 hi/lo planes interleaved per d-row: [D, 2, T]
    # (8KB contiguous DRAM rows -> max DMA packet efficiency)
    xt_d = nc.dram_tensor("xt", [D, 2, T], f16, kind="ExternalInput").ap()
    wkvh_d = nc.dram_tensor("wkvh", [P, ND * P], f16, kind="ExternalInput").ap()
    wkvl_d = nc.dram_tensor("wkvl", [P, ND * P], f16, kind="ExternalInput").ap()
    wqh_d = nc.dram_tensor("wqh", [P, ND * HS], f16, kind="ExternalInput").ap()
    wql_d = nc.dram_tensor("wql", [P, ND * HS], f16, kind="ExternalInput").ap()
    identb_d = nc.dram_tensor("identb", [P, P], f16, kind="ExternalInput").ap()
    mask_d = nc.dram_tensor("mask", [P, P], f32, kind="ExternalInput").ap()
    maskT_d = nc.dram_tensor("maskT", [P, P], f32, kind="ExternalInput").ap()
    # out^T per group: [65, 512] (64 head rows + Z row) x 2 groups
    out_d = nc.dram_tensor("out", [2 * 65, 512], f32, kind="ExternalOutput").ap()

    with tile.TileContext(nc) as tc:
        with tc.tile_pool(name="consts", bufs=1) as consts, \
             tc.tile_pool(name="big", bufs=1) as big:
            identb = consts.tile([P, P], f16)
            mask = consts.tile([P, P], f32)
            maskT = consts.tile([P, P], f32)
            wkvh = consts.tile([P, ND, P], f16)
            wkvl = consts.tile([P, ND, P], f16)
            wqh = consts.tile([P, ND, HS], f16)
            wql = consts.tile([P, ND, HS], f16)

            # kv weights first (gate the first matmul), then x tiles, then
            # the rest of the small consts
            nc.scalar.dma_start(wkvh[:], wkvh_d.rearrange("p (a h) -> p a h", a=ND))
            nc.scalar.dma_start(wkvl[:], wkvl_d.rearrange("p (a h) -> p a h", a=ND))

            # x^T tiles: one per dt, both planes: [128, 2, 2048] fp16,
            # single 1MB DMA with 8KB rows; alternate the two HWDGE queues.
            xt = [big.tile([P, 2, T], f16, name=f"xt{dt}", tag=f"xt{dt}")
                  for dt in range(ND)]
            for dt in range(ND):
                eng = (nc.scalar, nc.sync)[dt % 2]
                eng.dma_start(xt[dt][:], xt_d[dt * P:(dt + 1) * P])
            nc.scalar.dma_start(wqh[:], wqh_d.rearrange("p (a h) -> p a h", a=ND))
            nc.scalar.dma_start(wql[:], wql_d.rearrange("p (a h) -> p a h", a=ND))
            nc.scalar.dma_start(identb[:], identb_d[:])
            nc.scalar.dma_start(mask[:], mask_d[:])
            nc.scalar.dma_start(maskT[:], maskT_d[:])

            # KHL: rows 0:64 = k_lo, rows 64:128 = k_hi
            KHL = big.tile([P, T], f16)
            vTb = big.tile([HS, T], f16)
            # v natural + a ones column (row Z accumulator trick)
            vn2 = big.tile([P, NT, HS + 1], f16)
            qhh = big.tile([P, 2, 512], f16)   # rows 0:64=q_hi, 64:128=q_hi
            qlz = big.tile([P, 2, 512], f16)   # rows 0:64=0,    64:128=q_lo
            nc.vector.memset(qlz[0:HS, :, :], 0.0)
            nc.vector.memset(vn2[:, :, HS:HS + 1], 1.0)

            # ---- k,v projections over full T (both roles) ----
            with tc.tile_pool(name="pps", bufs=2, space="PSUM") as pps, \
                 tc.tile_pool(name="kltmp", bufs=2) as klt:
                for ch in range(NCH):
                    cs = slice(ch * 512, (ch + 1) * 512)
                    ps = pps.tile([P, 512], f32, tag="proj")
                    tsl = slice(ch * 512, (ch + 1) * 512)
                    i = 0
                    for w_t, pl in ((wkvh, 0), (wkvh, 1), (wkvl, 0)):
                        for dt in range(ND):
                            nc.tensor.matmul(
                                ps[:], lhsT=w_t[:, dt, :],
                                rhs=xt[dt][:, pl, tsl],
                                start=(i == 0), stop=(i == 23))
                            i += 1
                    # rows 0:64 = v^T, rows 64:128 = k
                    nc.scalar.copy(vTb[:, cs], ps[0:HS, :])
                    nc.scalar.copy(KHL[HS:P, cs], ps[HS:P, :])
                    kl = klt.tile([P, 512], f16, tag="kl")
                    nc.vector.tensor_sub(kl[HS:P, :], ps[HS:P, :],
                                         KHL[HS:P, cs])
                    nc.gpsimd.dma_start(KHL[0:HS, cs], kl[HS:P, :])
                    # v^T -> v natural (fp16) for this chunk's 4 key-blocks
                    for tt in range(4 * ch, 4 * ch + 4):
                        vp = pps.tile([P, HS], f16, tag="vre")
                        nc.tensor.transpose(
                            vp[:], vTb[:, tt * P:(tt + 1) * P],
                            identb[0:HS, 0:HS])
                        nc.scalar.copy(vn2[:, tt, 0:HS], vp[:])

            # ---- role-specific: q projections + attention (S^T-direct) ----
            with tc.tile_pool(name="spool", bufs=5, space="PSUM") as spool, \
                 tc.tile_pool(name="avp", bufs=2, space="PSUM") as avp, \
                 tc.tile_pool(name="btp", bufs=1, space="PSUM") as btp, \
                 tc.tile_pool(name="ets", bufs=16) as ets, \
                 tc.tile_pool(name="bfp", bufs=2) as bfp, \
                 tc.tile_pool(name="small", bufs=4) as small, \
                 tc.tile_pool(name="osb", bufs=2) as osb, \
                 tc.tile_pool(name="qtmp", bufs=2) as qtp:

                def emit_role(role):
                    for qc, ch in enumerate(ROLE_QCHUNKS[role]):
                        ps = spool.tile([HS, 512], f32, tag="S")
                        tsl = slice(ch * 512, (ch + 1) * 512)
                        i = 0
                        for w_t, pl in ((wqh, 0), (wqh, 1), (wql, 0)):
                            for dt in range(ND):
                                nc.tensor.matmul(
                                    ps[:], lhsT=w_t[:, dt, :],
                                    rhs=xt[dt][:, pl, tsl],
                                    start=(i == 0), stop=(i == 23))
                                i += 1
                        nc.scalar.copy(qhh[0:HS, qc, :], ps[:])
                        qt = qtp.tile([HS, 512], f16, tag="qt")
                        nc.vector.tensor_sub(qt[:], ps[:], qhh[0:HS, qc, :])
                        nc.gpsimd.dma_start(qhh[HS:P, qc, :], qhh[0:HS, qc, :])
                        nc.gpsimd.dma_start(qlz[HS:P, qc, :], qt[:])

                    groups = [ROLE_BLOCKS[role][0:4], ROLE_BLOCKS[role][4:8]]

                    # per group: approx row-max pass (fp16 hi x hi,
                    # partitions 64:128), then the S^T/exp/AV loop —
                    # interleaved so one group's DVE reduces don't block
                    # the other group's softmax chain on the strict FIFO
                    for g, blocks in enumerate(groups):
                        ball = small.tile([P, 4], f16, tag=f"ball{g}")
                        for bi, j in enumerate(blocks):
                            L = 128 * (j + 1)
                            nch = (L + 511) // 512
                            off = bi * P
                            mc = small.tile([P, 4], f32, tag="mc")
                            for kc in range(nch):
                                w = min(512, L - kc * 512)
                                sap = spool.tile([P, w], f32, tag="S")
                                nc.tensor.matmul(
                                    sap[:], lhsT=qhh[HS:P, g, off:off + P],
                                    rhs=KHL[HS:P, kc * 512:kc * 512 + w],
                                    start=True, stop=True)
                                if kc == nch - 1:
                                    nc.vector.tensor_add(
                                        sap[:, w - P:w], sap[:, w - P:w],
                                        mask[:])
                                nc.vector.reduce_max(
                                    mc[:, kc:kc + 1], sap[:], axis=AX)
                            m = small.tile([P, 1], f32, tag="m")
                            if nch == 1:
                                nc.vector.tensor_scalar_add(
                                    ball[:, bi:bi + 1], mc[:, 0:1], 0.125)
                            else:
                                nc.vector.reduce_max(m[:], mc[:, 0:nch],
                                                     axis=AX)
                                nc.vector.tensor_scalar_add(
                                    ball[:, bi:bi + 1], m[:], 0.125)
                        # B columns -> row layout: [128,4] -> [4,128] ->
                        # [1,512] -> broadcast to [128,512]
                        bt = btp.tile([4, P], f16, tag="bt")
                        nc.tensor.transpose(bt[:], ball[:], identb[:])
                        bts = small.tile([4, P], f16, tag="bts")
                        nc.vector.tensor_copy(bts[:], bt[:])
                        brow = small.tile([1, 512], f16, tag="brow")
                        nc.gpsimd.dma_start(
                            brow[0:1, :].rearrange("a (b c) -> a b c", b=4),
                            bts[:])
                        bfull = bfp.tile([P, 512], f16, tag="bfull")
                        nc.gpsimd.partition_broadcast(bfull[:], brow[:])

                        nkt = blocks[-1] + 1
                        avg = avp.tile([HS + 1, 512], f32, tag="avg")
                        et_tiles = []
                        for kt in range(nkt):
                            sp = spool.tile([P, 512], f32, tag="S")
                            kb = KHL[:, kt * P:(kt + 1) * P]
                            nc.tensor.matmul(sp[:], lhsT=kb,
                                             rhs=qhh[:, g, :],
                                             start=True, stop=False)
                            nc.tensor.matmul(sp[:], lhsT=kb,
                                             rhs=qlz[:, g, :],
                                             start=False, stop=True)
                            if kt >= blocks[0]:
                                bi = kt - blocks[0]
                                nc.vector.tensor_add(
                                    sp[:, bi * P:(bi + 1) * P],
                                    sp[:, bi * P:(bi + 1) * P], maskT[:])
                            et = ets.tile([P, 512], f16, tag="et")
                            nc.vector.tensor_sub(et[:], sp[:], bfull[:])
                            nc.scalar.activation(et[:], et[:], EXP,
                                                 scale=SCALE)
                            et_tiles.append(et)
                            # AV for blocks whose causal span just completed,
                            # keeping accumulation groups sequential per tile
                            for bi, j in enumerate(blocks):
                                if j == kt:
                                    for ki in range(j + 1):
                                        nc.tensor.matmul(
                                            avg[:, bi * P:(bi + 1) * P],
                                            lhsT=vn2[:, ki, :],
                                            rhs=et_tiles[ki][
                                                :, bi * P:(bi + 1) * P],
                                            start=(ki == 0), stop=(ki == j),
                                            skip_group_check=True)
                        avs = osb.tile([HS + 1, 512], f32, tag="avs")
                        nc.vector.tensor_copy(avs[:], avg[:])
                        nc.sync.dma_start(out_d[g * 65:(g + 1) * 65, :],
                                          avs[:])

                pid = nc.partition_id()
                with tc.If(pid < 4) as cmp:
                    emit_role(0)
                with cmp.Else():
                    emit_role(1)

    nc.compile()
    return nc


def _get_program():
    global _COMPILED
    if _COMPILED is None:
        _COMPILED = _build()
    return _COMPILED


def _install_ntff_hook():
    import sys, types
    if "antenv.axon_hooks" in sys.modules:
        return
    try:
        from trn_agent_boot.trn_boot import _ntff_profile_via_ctypes
        hook = _ntff_profile_via_ctypes("/opt/axon/libaxon_pjrt.so")
        mod = types.ModuleType("antenv.axon_hooks")
        mod.get_axon_ntff_profile_hook = lambda: hook
        mod.set_axon_ntff_profile_hook = lambda h: None
        import antenv
        sys.modules["antenv.axon_hooks"] = mod
        antenv.axon_hooks = mod
    except Exception:
        pass


def _split_pair16(a):
    hi = a.astype(np.float16)
    lo = (a - hi.astype(np.float32)).astype(np.float16)
    return hi, lo


def _host_prep(inputs):
    x = np.asarray(inputs["x"], dtype=np.float32)
    wq = np.asarray(inputs["Wq"], dtype=np.float32)
    wk = np.asarray(inputs["Wk"], dtype=np.float32)
    wv = np.asarray(inputs["Wv"], dtype=np.float32)

    xtf = np.ascontiguousarray(np.transpose(x, (0, 2, 1)))  # [B, D, T] f32
    xh, xl = _split_pair16(xtf)
    xt = np.stack([xh, xl], axis=2)                # [B, D, 2, T] fp16

    def _wprep(wt):
        # [D, M] -> [P, ND*M]: row p holds [dt, m] for d = dt*P + p
        m = wt.shape[1]
        return np.ascontiguousarray(
            wt.reshape(ND, P, m).transpose(1, 0, 2).reshape(P, ND * m))

    wkvT = np.concatenate([wv, wk], axis=0).T      # [D, 128]
    wkvh, wkvl = _split_pair16(_wprep(wkvT))
    wqT = wq.T                                     # [D, 64]
    wqh, wql = _split_pair16(_wprep(wqT))

    identb = np.eye(P, dtype=np.float16)
    r = np.arange(P)
    # mask[q, k]: masked (NEG) where k > q; maskT is its transpose
    mask = np.where(r[None, :] <= r[:, None], 0.0, NEG).astype(np.float32)
    maskT = np.ascontiguousarray(mask.T)

    shared = {"wkvh": wkvh, "wkvl": wkvl, "wqh": wqh, "wql": wql,
              "identb": identb, "mask": mask, "maskT": maskT}
    in_maps = []
    for c in range(N_CORES):
        m = dict(shared)
        m["xt"] = np.ascontiguousarray(xt[c % B])
        in_maps.append(m)
    return in_maps


def _run(inputs, trace=False):
    from concourse.bass_utils import run_bass_kernel_spmd

    if trace:
        _install_ntff_hook()
    nc = _get_program()
    in_maps = _host_prep(inputs)
    res = run_bass_kernel_spmd(nc, in_maps, list(range(N_CORES)), trace=trace)

    out = np.empty((B, T, HS), dtype=np.float32)
    for c in range(N_CORES):
        b, role = c % B, c // B
        oc = res.results[c]["out"]          # [130, 512]: 2 groups x [65, 512]
        for g in range(2):
            blocks = ROLE_BLOCKS[role][4 * g:4 * g + 4]
            avt = oc[65 * g:65 * g + 65]
            for bi, j in enumerate(blocks):
                sub = avt[0:HS, 128 * bi:128 * (bi + 1)]
                z = avt[HS:HS + 1, 128 * bi:128 * (bi + 1)]
                out[b, 128 * j:128 * (j + 1)] = (sub / z).T
    return out, res


def kernel(**inputs):
    out, _ = _run(inputs, trace=False)
    return out


# revision 55
# speedup vs baseline: 1.0408x; 1.0408x over previous
"""Single-head causal attention (B=4, T=2048, D=1024, HS=64) on 8 TRN2 cores.

Sharding: 2 cores per batch element. Query blocks (128 rows, 16/batch) are
fold-split for causal balance:
  role 0 (cores 0-3): blocks {0,1,2,3,12,13,14,15} of batch (core_id % 4)
  role 1 (cores 4-7): blocks {4..11}              of batch (core_id % 4)

Precision scheme (fp16, 10 mantissa bits):
  host: x.T in fp16 single plane (4MB/batch DMA), W pre-transposed fp16
  hi/lo pairs. k,v,q projections: 2 matmul groups (x*wh + x*wl) in fp32
  PSUM — error dominated by the single fp16 rounding of x (~2^-11 rel).
  scores: k,q re-split into fp16 hi/lo pairs on device;
    S = [qh;qh]^T.[kl;kh] + [0;ql]^T.[kl;kh]  (2 matmuls per 512-chunk)
  softmax: chunked row-max (DVE) + exp on ACT (scale=8, bias=-8*max),
  E fp16; E^T via PE transposes batched 4-per-PSUM-tile; AV inverted:
  out[q,h] accumulates lhsT=E^T-block (stationary, FWL) x rhs=v-natural
  so the result lands layout-natural and 1/Z applies per-partition.
"""

import numpy as np

N_CORES = 8
B, T, D, HS = 4, 2048, 1024, 64
P = 128
NT = T // P        # 16
ND = D // P        # 8
NCH = 4            # 512-wide t chunks
SCALE = 8.0        # sqrt(HS)
NEG = -1.0e30

ROLE_BLOCKS = [
    [0, 1, 2, 3, 12, 13, 14, 15],
    [4, 5, 6, 7, 8, 9, 10, 11],
]
ROLE_QCHUNKS = [[0, 3], [1, 2]]  # 512-chunk indices holding each role's q rows


def _block_qloc(role, j):
    if role == 0:
        return (0, 128 * j) if j < 4 else (1, 128 * (j - 12))
    return (0, 128 * (j - 4)) if j < 8 else (1, 128 * (j - 8))


_COMPILED = None


def _build():
    import concourse.bass as bass
    import concourse.tile as tile
    from concourse import bacc, mybir

    f32 = mybir.dt.float32
    f16 = mybir.dt.float16
    EXP = mybir.ActivationFunctionType.Exp
    AX = mybir.AxisListType.X

    nc = bacc.Bacc("TRN2", target_bir_lowering=False, debug=False,
                   num_devices=N_CORES)

    # x^T per batch, fp16 hi/lo planes interleaved per d-row: [D, 2, T]
    # (8KB contiguous DRAM rows -> max DMA packet efficiency)
    xt_d = nc.dram_tensor("xt", [D, 2, T], f16, kind="ExternalInput").ap()
    wkvh_d = nc.dram_tensor("wkvh", [P, ND * P], f16, kind="ExternalInput").ap()
    wkvl_d = nc.dram_tensor("wkvl", [P, ND * P], f16, kind="ExternalInput").ap()
    wqh_d = nc.dram_tensor("wqh", [P, ND * HS], f16, kind="ExternalInput").ap()
    wql_d = nc.dram_tensor("wql", [P, ND * HS], f16, kind="ExternalInput").ap()
    identb_d = nc.dram_tensor("identb", [P, P], f16, kind="ExternalInput").ap()
    mask_d = nc.dram_tensor("mask", [P, P], f32, kind="ExternalInput").ap()
    maskT_d = nc.dram_tensor("maskT", [P, P], f32, kind="ExternalInput").ap()
    # out^T per group: [65, 512] (64 head rows + Z row) x 2 groups
    out_d = nc.dram_tensor("out", [2 * 65, 512], f32, kind="ExternalOutput").ap()

    with tile.TileContext(nc) as tc:
        with tc.tile_pool(name="consts", bufs=1) as consts, \
             tc.tile_pool(name="big", bufs=1) as big:
            identb = consts.tile([P, P], f16)
            mask = consts.tile([P, P], f32)
            maskT = consts.tile([P, P], f32)
            wkvh = consts.tile([P, ND, P], f16)
            wkvl = consts.tile([P, ND, P], f16)
            wqh = consts.tile([P, ND, HS], f16)
            wql = consts.tile([P, ND, HS], f16)

            # kv weights first (gate the first matmul), then x tiles, then
            # the rest of the small consts
            nc.scalar.dma_start(wkvh[:], wkvh_d.rearrange("p (a h) -> p a h", a=ND))

            # x^T tiles: one per dt, both planes: [128, 2, 2048] fp16,
            # single 1MB DMA with 8KB rows; alternate the two HWDGE queues.
            xt = [big.tile([P, 2, T], f16, name=f"xt{dt}", tag=f"xt{dt}")
                  for dt in range(ND)]
            for dt in range(ND):
                eng = (nc.scalar, nc.sync)[dt % 2]
                eng.dma_start(xt[dt][:], xt_d[dt * P:(dt + 1) * P])
                if dt == 0:
                    nc.scalar.dma_start(
                        wkvl[:], wkvl_d.rearrange("p (a h) -> p a h", a=ND))
            nc.scalar.dma_start(wqh[:], wqh_d.rearrange("p (a h) -> p a h", a=ND))
            nc.scalar.dma_start(wql[:], wql_d.rearrange("p (a h) -> p a h", a=ND))
            nc.scalar.dma_start(identb[:], identb_d[:])
            nc.scalar.dma_start(mask[:], mask_d[:])
            nc.scalar.dma_start(maskT[:], maskT_d[:])

            # KHL: rows 0:64 = k_lo, rows 64:128 = k_hi
            KHL = big.tile([P, T], f16)
            vTb = big.tile([HS, T], f16)
            # v natural + a ones column (row Z accumulator trick)
            vn2 = big.tile([P, NT, HS + 1], f16)
            qhh = big.tile([P, 2, 512], f16)   # rows 0:64=q_hi, 64:128=q_hi
            qlz = big.tile([P, 2, 512], f16)   # rows 0:64=0,    64:128=q_lo
            negones = big.tile([1, P], f16)
            nc.vector.memset(qlz[0:HS, :, :], 0.0)
            nc.vector.memset(vn2[:, :, HS:HS + 1], 1.0)
            nc.vector.memset(negones[:], -1.0)

            # ---- k,v projections over full T (both roles) ----
            with tc.tile_pool(name="pps", bufs=2, space="PSUM") as pps, \
                 tc.tile_pool(name="kltmp", bufs=2) as klt:
                for ch in range(NCH):
                    cs = slice(ch * 512, (ch + 1) * 512)
                    ps = pps.tile([P, 512], f32, tag="proj")
                    tsl = slice(ch * 512, (ch + 1) * 512)
                    i = 0
                    for w_t, pl in ((wkvh, 0), (wkvh, 1), (wkvl, 0)):
                        for dt in range(ND):
                            nc.tensor.matmul(
                                ps[:], lhsT=w_t[:, dt, :],
                                rhs=xt[dt][:, pl, tsl],
                                start=(i == 0), stop=(i == 23))
                            i += 1
                    # rows 0:64 = v^T, rows 64:128 = k
                    nc.scalar.copy(vTb[:, cs], ps[0:HS, :])
                    nc.scalar.copy(KHL[HS:P, cs], ps[HS:P, :])
                    kl = klt.tile([P, 512], f16, tag="kl")
                    nc.vector.tensor_sub(kl[HS:P, :], ps[HS:P, :],
                                         KHL[HS:P, cs])
                    nc.gpsimd.dma_start(KHL[0:HS, cs], kl[HS:P, :])
                    # v^T -> v natural (fp16) for this chunk's 4 key-blocks
                    for tt in range(4 * ch, 4 * ch + 4):
                        vp = pps.tile([P, HS], f16, tag="vre")
                        nc.tensor.transpose(
                            vp[:], vTb[:, tt * P:(tt + 1) * P],
                            identb[0:HS, 0:HS])
                        nc.scalar.copy(vn2[:, tt, 0:HS], vp[:])

            # ---- role-specific: q projections + attention (S^T-direct) ----
            with tc.tile_pool(name="spool", bufs=5, space="PSUM") as spool, \
                 tc.tile_pool(name="avp", bufs=2, space="PSUM") as avp, \
                 tc.tile_pool(name="btp", bufs=1, space="PSUM") as btp, \
                 tc.tile_pool(name="ets", bufs=16) as ets, \
                 tc.tile_pool(name="bfp", bufs=2) as bfp, \
                 tc.tile_pool(name="small", bufs=4) as small, \
                 tc.tile_pool(name="osb", bufs=2) as osb, \
                 tc.tile_pool(name="qtmp", bufs=2) as qtp:

                def emit_role(role):
                    for qc, ch in enumerate(ROLE_QCHUNKS[role]):
                        ps = spool.tile([HS, 512], f32, tag="S")
                        tsl = slice(ch * 512, (ch + 1) * 512)
                        i = 0
                        for w_t, pl in ((wqh, 0), (wqh, 1), (wql, 0)):
                            for dt in range(ND):
                                nc.tensor.matmul(
                                    ps[:], lhsT=w_t[:, dt, :],
                                    rhs=xt[dt][:, pl, tsl],
                                    start=(i == 0), stop=(i == 23))
                                i += 1
                        nc.scalar.copy(qhh[0:HS, qc, :], ps[:])
                        qt = qtp.tile([HS, 512], f16, tag="qt")
                        nc.vector.tensor_sub(qt[:], ps[:], qhh[0:HS, qc, :])
                        nc.gpsimd.dma_start(qhh[HS:P, qc, :], qhh[0:HS, qc, :])
                        nc.gpsimd.dma_start(qlz[HS:P, qc, :], qt[:])

                    groups = [ROLE_BLOCKS[role][0:4], ROLE_BLOCKS[role][4:8]]

                    # per group: approx row-max pass (fp16 hi x hi,
                    # partitions 64:128), then the S^T/exp/AV loop —
                    # interleaved so one group's DVE reduces don't block
                    # the other group's softmax chain on the strict FIFO
                    for g, blocks in enumerate(groups):
                        ball = small.tile([P, 4], f16, tag=f"ball{g}")
                        for bi, j in enumerate(blocks):
                            L = 128 * (j + 1)
                            nch = (L + 511) // 512
                            off = bi * P
                            mc = small.tile([P, 4], f32, tag="mc")
                            for kc in range(nch):
                                w = min(512, L - kc * 512)
                                sap = spool.tile([P, w], f32, tag="S")
                                nc.tensor.matmul(
                                    sap[:], lhsT=qhh[HS:P, g, off:off + P],
                                    rhs=KHL[HS:P, kc * 512:kc * 512 + w],
                                    start=True, stop=True)
                                if kc == nch - 1:
                                    nc.vector.tensor_add(
                                        sap[:, w - P:w], sap[:, w - P:w],
                                        mask[:])
                                nc.vector.reduce_max(
                                    mc[:, kc:kc + 1], sap[:], axis=AX)
                            m = small.tile([P, 1], f32, tag="m")
                            if nch == 1:
                                nc.vector.tensor_scalar_add(
                                    ball[:, bi:bi + 1], mc[:, 0:1], 0.125)
                            else:
                                nc.vector.reduce_max(m[:], mc[:, 0:nch],
                                                     axis=AX)
                                nc.vector.tensor_scalar_add(
                                    ball[:, bi:bi + 1], m[:], 0.125)
                        # B columns -> row layout: [128,4] -> [4,128] ->
                        # [1,512]; bias lands in PSUM via -ones x brow
                        bt = btp.tile([4, P], f16, tag="bt")
                        nc.tensor.transpose(bt[:], ball[:], identb[:])
                        bts = small.tile([4, P], f16, tag="bts")
                        nc.scalar.copy(bts[:], bt[:])
                        brow = bfp.tile([1, 512], f16, tag="brow")
                        nc.gpsimd.dma_start(
                            brow[0:1, :].rearrange("a (b c) -> a b c", b=4),
                            bts[:])

                        nkt = blocks[-1] + 1
                        avg = avp.tile([HS + 1, 512], f32, tag="avg")
                        et_tiles = []
                        for kt in range(nkt):
                            sp = spool.tile([P, 512], f32, tag="S")
                            kb = KHL[:, kt * P:(kt + 1) * P]
                            nc.tensor.matmul(sp[:], lhsT=kb,
                                             rhs=qhh[:, g, :],
                                             start=True, stop=False)
                            nc.tensor.matmul(sp[:], lhsT=kb,
                                             rhs=qlz[:, g, :],
                                             start=False, stop=False)
                            nc.tensor.matmul(sp[:], lhsT=negones[:],
                                             rhs=brow[:],
                                             start=False, stop=True,
                                             skip_group_check=True)
                            if kt >= blocks[0]:
                                bi = kt - blocks[0]
                                nc.vector.tensor_add(
                                    sp[:, bi * P:(bi + 1) * P],
                                    sp[:, bi * P:(bi + 1) * P], maskT[:])
                            et = ets.tile([P, 512], f16, tag="et")
                            nc.scalar.activation(et[:], sp[:], EXP,
                                                 scale=SCALE)
                            et_tiles.append(et)
                            # AV for blocks whose causal span just completed,
                            # keeping accumulation groups sequential per tile
                            for bi, j in enumerate(blocks):
                                if j == kt:
                                    for ki in range(j + 1):
                                        nc.tensor.matmul(
                                            avg[:, bi * P:(bi + 1) * P],
                                            lhsT=vn2[:, ki, :],
                                            rhs=et_tiles[ki][
                                                :, bi * P:(bi + 1) * P],
                                            start=(ki == 0), stop=(ki == j),
                                            skip_group_check=True)
                        avs = osb.tile([HS + 1, 512], f32, tag="avs")
                        nc.vector.tensor_copy(avs[:], avg[:])
                        nc.sync.dma_start(out_d[g * 65:(g + 1) * 65, :],
                                          avs[:])

                pid = nc.partition_id()
                with tc.If(pid < 4) as cmp:
                    emit_role(0)
                with cmp.Else():
                    emit_role(1)

    nc.compile()
    return nc


def _get_program():
    global _COMPILED
    if _COMPILED is None:
        _COMPILED = _build()
    return _COMPILED


def _install_ntff_hook():
    import sys, types
    if "antenv.axon_hooks" in sys.modules:
        return
    try:
        from trn_agent_boot.trn_boot import _ntff_profile_via_ctypes
        hook = _ntff_profile_via_ctypes("/opt/axon/libaxon_pjrt.so")
        mod = types.ModuleType("antenv.axon_hooks")
        mod.get_axon_ntff_profile_hook = lambda: hook
        mod.set_axon_ntff_profile_hook = lambda h: None
        import antenv
        sys.modules["antenv.axon_hooks"] = mod
        antenv.axon_hooks = mod
    except Exception:
        pass


def _split_pair16(a):
    hi = a.astype(np.float16)
    lo = (a - hi.astype(np.float32)).astype(np.float16)
    return hi, lo


def _host_prep(inputs):
    x = np.asarray(inputs["x"], dtype=np.float32)
    wq = np.asarray(inputs["Wq"], dtype=np.float32)
    wk = np.asarray(inputs["Wk"], dtype=np.float32)
    wv = np.asarray(inputs["Wv"], dtype=np.float32)

    xtf = np.ascontiguousarray(np.transpose(x, (0, 2, 1)))  # [B, D, T] f32
    xh, xl = _split_pair16(xtf)
    xt = np.stack([xh, xl], axis=2)                # [B, D, 2, T] fp16

    def _wprep(wt):
        # [D, M] -> [P, ND*M]: row p holds [dt, m] for d = dt*P + p
        m = wt.shape[1]
        return np.ascontiguousarray(
            wt.reshape(ND, P, m).transpose(1, 0, 2).reshape(P, ND * m))

    wkvT = np.concatenate([wv, wk], axis=0).T      # [D, 128]
    wkvh, wkvl = _split_pair16(_wprep(wkvT))
    wqT = wq.T                                     # [D, 64]
    wqh, wql = _split_pair16(_wprep(wqT))

    identb = np.eye(P, dtype=np.float16)
    r = np.arange(P)
    # mask[q, k]: masked (NEG) where k > q; maskT is its transpose
    mask = np.where(r[None, :] <= r[:, None], 0.0, NEG).astype(np.float32)
    maskT = np.ascontiguousarray(mask.T)

    shared = {"wkvh": wkvh, "wkvl": wkvl, "wqh": wqh, "wql": wql,
              "identb": identb, "mask": mask, "maskT": maskT}
    in_maps = []
    for c in range(N_CORES):
        m = dict(shared)
        m["xt"] = np.ascontiguousarray(xt[c % B])
        in_maps.append(m)
    return in_maps


def _run(inputs, trace=False):
    from concourse.bass_utils import run_bass_kernel_spmd

    if trace:
        _install_ntff_hook()
    nc = _get_program()
    in_maps = _host_prep(inputs)
    res = run_bass_kernel_spmd(nc, in_maps, list(range(N_CORES)), trace=trace)

    out = np.empty((B, T, HS), dtype=np.float32)
    for c in range(N_CORES):
        b, role = c % B, c // B
        oc = res.results[c]["out"]          # [130, 512]: 2 groups x [65, 512]
        for g in range(2):
            blocks = ROLE_BLOCKS[role][4 * g:4 * g + 4]
            avt = oc[65 * g:65 * g + 65]
            for bi, j in enumerate(blocks):
                sub = avt[0:HS, 128 * bi:128 * (bi + 1)]
                z = avt[HS:HS + 1, 128 * bi:128 * (bi + 1)]
                out[b, 128 * j:128 * (j + 1)] = (sub / z).T
    return out, res


def kernel(**inputs):
    out, _ = _run(inputs, trace=False)
    return out


# revision 58
# speedup vs baseline: 1.0508x; 1.0097x over previous
"""Single-head causal attention (B=4, T=2048, D=1024, HS=64) on 8 TRN2 cores.

Sharding: 2 cores per batch element. Query blocks (128 rows, 16/batch) are
fold-split for causal balance:
  role 0 (cores 0-3): blocks {0,1,2,3,12,13,14,15} of batch (core_id % 4)
  role 1 (cores 4-7): blocks {4..11}              of batch (core_id % 4)

Precision scheme (fp16, 10 mantissa bits):
  host: x.T in fp16 single plane (4MB/batch DMA), W pre-transposed fp16
  hi/lo pairs. k,v,q projections: 2 matmul groups (x*wh + x*wl) in fp32
  PSUM — error dominated by the single fp16 rounding of x (~2^-11 rel).
  scores: k,q re-split into fp16 hi/lo pairs on device;
    S = [qh;qh]^T.[kl;kh] + [0;ql]^T.[kl;kh]  (2 matmuls per 512-chunk)
  softmax: chunked row-max (DVE) + exp on ACT (scale=8, bias=-8*max),
  E fp16; E^T via PE transposes batched 4-per-PSUM-tile; AV inverted:
  out[q,h] accumulates lhsT=E^T-block (stationary, FWL) x rhs=v-natural
  so the result lands layout-natural and 1/Z applies per-partition.
"""

import numpy as np

N_CORES = 8
B, T, D, HS = 4, 2048, 1024, 64
P = 128
NT = T // P        # 16
ND = D // P        # 8
NCH = 4            # 512-wide t chunks
SCALE = 8.0        # sqrt(HS)
NEG = -1.0e30

ROLE_BLOCKS = [
    [0, 1, 2, 3, 12, 13, 14, 15],
    [4, 5, 6, 7, 8, 9, 10, 11],
]
ROLE_QCHUNKS = [[0, 3], [1, 2]]  # 512-chunk indices holding each role's q rows


def _block_qloc(role, j):
    if role == 0:
        return (0, 128 * j) if j < 4 else (1, 128 * (j - 12))
    return (0, 128 * (j - 4)) if j < 8 else (1, 128 * (j - 8))


_COMPILED = None


def _build():
    import concourse.bass as bass
    import concourse.tile as tile
    from concourse import bacc, mybir

    f32 = mybir.dt.float32
    f16 = mybir.dt.float16
    EXP = mybir.ActivationFunctionType.Exp
    AX = mybir.AxisListType.X

    nc = bacc.Bacc("TRN2", target_bir_lowering=False, debug=False,
                   num_devices=N_CORES)

    # x^T per batch, fp16 hi/lo planes interleaved per d-row: [D, 2, T]
    # (8KB contiguous DRAM rows -> max DMA packet efficiency)
    xt_d = nc.dram_tensor("xt", [D, 2, T], f16, kind="ExternalInput").ap()
    wkvh_d = nc.dram_tensor("wkvh", [P, ND * P], f16, kind="ExternalInput").ap()
    wkvl_d = nc.dram_tensor("wkvl", [P, ND * P], f16, kind="ExternalInput").ap()
    wqh_d = nc.dram_tensor("wqh", [P, ND * HS], f16, kind="ExternalInput").ap()
    wql_d = nc.dram_tensor("wql", [P, ND * HS], f16, kind="ExternalInput").ap()
    identb_d = nc.dram_tensor("identb", [P, P], f16, kind="ExternalInput").ap()
    mask_d = nc.dram_tensor("mask", [P, P], f32, kind="ExternalInput").ap()
    maskT_d = nc.dram_tensor("maskT", [P, P], f32, kind="ExternalInput").ap()
    # out^T per group: [65, 512] (64 head rows + Z row) x 2 groups
    out_d = nc.dram_tensor("out", [2 * 65, 512], f32, kind="ExternalOutput").ap()

    with tile.TileContext(nc) as tc:
        with tc.tile_pool(name="consts", bufs=1) as consts, \
             tc.tile_pool(name="big", bufs=1) as big:
            identb = consts.tile([P, P], f16)
            mask = consts.tile([P, P], f32)
            maskT = consts.tile([P, P], f32)
            wkvh = consts.tile([P, ND, P], f16)
            wkvl = consts.tile([P, ND, P], f16)
            wqh = consts.tile([P, ND, HS], f16)
            wql = consts.tile([P, ND, HS], f16)

            # kv weights first (gate the first matmul), then x tiles, then
            # the rest of the small consts
            nc.scalar.dma_start(wkvh[:], wkvh_d.rearrange("p (a h) -> p a h", a=ND))

            # x^T tiles: one per dt, both planes: [128, 2, 2048] fp16,
            # single 1MB DMA with 8KB rows; alternate the two HWDGE queues.
            xt = [big.tile([P, 2, T], f16, name=f"xt{dt}", tag=f"xt{dt}")
                  for dt in range(ND)]
            for dt in range(ND):
                eng = (nc.scalar, nc.sync)[dt % 2]
                eng.dma_start(xt[dt][:], xt_d[dt * P:(dt + 1) * P])
                if dt == 0:
                    nc.scalar.dma_start(
                        wkvl[:], wkvl_d.rearrange("p (a h) -> p a h", a=ND))
            nc.scalar.dma_start(wqh[:], wqh_d.rearrange("p (a h) -> p a h", a=ND))
            nc.scalar.dma_start(wql[:], wql_d.rearrange("p (a h) -> p a h", a=ND))
            nc.scalar.dma_start(identb[:], identb_d[:])
            nc.scalar.dma_start(mask[:], mask_d[:])
            nc.scalar.dma_start(maskT[:], maskT_d[:])

            # KHL: rows 0:64 = k_lo, rows 64:128 = k_hi
            KHL = big.tile([P, T], f16)
            vTb = big.tile([HS, T], f16)
            # v natural + a ones column (row Z accumulator trick)
            vn2 = big.tile([P, NT, HS + 1], f16)
            qhh = big.tile([P, 2, 512], f16)   # rows 0:64=q_hi, 64:128=q_hi
            qlz = big.tile([P, 2, 512], f16)   # rows 0:64=0,    64:128=q_lo
            negones = big.tile([1, P], f16)
            nc.vector.memset(qlz[0:HS, :, :], 0.0)
            nc.vector.memset(vn2[:, :, HS:HS + 1], 1.0)
            nc.vector.memset(negones[:], -1.0)

            # ---- k,v projections over full T (both roles) ----
            with tc.tile_pool(name="pps", bufs=2, space="PSUM") as pps, \
                 tc.tile_pool(name="kltmp", bufs=2) as klt:
                # HAM warm-up: dummy matmuls on the first-arrived weight tile
                # while the x DMA streams in, so real matmuls start at 2.4GHz
                wps = pps.tile([P, P], f32, tag="warm")
                for _ in range(40):
                    nc.tensor.matmul(wps[:], lhsT=wkvh[:, 0, :],
                                     rhs=wkvh[:, 0, :], start=True, stop=True)
                for ch in range(NCH):
                    cs = slice(ch * 512, (ch + 1) * 512)
                    ps = pps.tile([P, 512], f32, tag="proj")
                    tsl = slice(ch * 512, (ch + 1) * 512)
                    i = 0
                    for w_t, pl in ((wkvh, 0), (wkvh, 1), (wkvl, 0)):
                        for dt in range(ND):
                            nc.tensor.matmul(
                                ps[:], lhsT=w_t[:, dt, :],
                                rhs=xt[dt][:, pl, tsl],
                                start=(i == 0), stop=(i == 23))
                            i += 1
                    # rows 0:64 = v^T, rows 64:128 = k
                    nc.scalar.copy(vTb[:, cs], ps[0:HS, :])
                    nc.scalar.copy(KHL[HS:P, cs], ps[HS:P, :])
                    kl = klt.tile([P, 512], f16, tag="kl")
                    nc.vector.tensor_sub(kl[HS:P, :], ps[HS:P, :],
                                         KHL[HS:P, cs])
                    nc.gpsimd.dma_start(KHL[0:HS, cs], kl[HS:P, :])
                    # v^T -> v natural (fp16) for this chunk's 4 key-blocks
                    for tt in range(4 * ch, 4 * ch + 4):
                        vp = pps.tile([P, HS], f16, tag="vre")
                        nc.tensor.transpose(
                            vp[:], vTb[:, tt * P:(tt + 1) * P],
                            identb[0:HS, 0:HS])
                        nc.scalar.copy(vn2[:, tt, 0:HS], vp[:])

            # ---- role-specific: q projections + attention (S^T-direct) ----
            with tc.tile_pool(name="spool", bufs=5, space="PSUM") as spool, \
                 tc.tile_pool(name="avp", bufs=2, space="PSUM") as avp, \
                 tc.tile_pool(name="btp", bufs=1, space="PSUM") as btp, \
                 tc.tile_pool(name="ets", bufs=16) as ets, \
                 tc.tile_pool(name="bfp", bufs=2) as bfp, \
                 tc.tile_pool(name="small", bufs=4) as small, \
                 tc.tile_pool(name="osb", bufs=2) as osb, \
                 tc.tile_pool(name="qtmp", bufs=2) as qtp:

                def emit_role(role):
                    for qc, ch in enumerate(ROLE_QCHUNKS[role]):
                        ps = spool.tile([HS, 512], f32, tag="S")
                        tsl = slice(ch * 512, (ch + 1) * 512)
                        i = 0
                        for w_t, pl in ((wqh, 0), (wqh, 1), (wql, 0)):
                            for dt in range(ND):
                                nc.tensor.matmul(
                                    ps[:], lhsT=w_t[:, dt, :],
                                    rhs=xt[dt][:, pl, tsl],
                                    start=(i == 0), stop=(i == 23))
                                i += 1
                        nc.scalar.copy(qhh[0:HS, qc, :], ps[:])
                        qt = qtp.tile([HS, 512], f16, tag="qt")
                        nc.vector.tensor_sub(qt[:], ps[:], qhh[0:HS, qc, :])
                        nc.gpsimd.dma_start(qhh[HS:P, qc, :], qhh[0:HS, qc, :])
                        nc.gpsimd.dma_start(qlz[HS:P, qc, :], qt[:])

                    # big group (blocks 4-7 of the role) runs FIRST: its
                    # diagonal mask-adds land only in the last 4 kts, so the
                    # small group's approx reduces weave into the mask-free
                    # stretch; small group last also shrinks the tail.
                    groups = [(1, ROLE_BLOCKS[role][4:8]),
                              (0, ROLE_BLOCKS[role][0:4])]
                    balls = {g: small.tile([P, 4], f16, tag=f"ball{g}",
                                           name=f"ball{g}")
                             for g, _ in groups}

                    def approx_block(g, blocks, bi):
                        j = blocks[bi]
                        L = 128 * (j + 1)
                        nch = (L + 511) // 512
                        off = bi * P
                        mc = small.tile([P, 4], f32, tag="mc", name="mc")
                        for kc in range(nch):
                            w = min(512, L - kc * 512)
                            sap = spool.tile([P, w], f32, tag="S",
                                             name="sap")
                            nc.tensor.matmul(
                                sap[:], lhsT=qhh[HS:P, g, off:off + P],
                                rhs=KHL[HS:P, kc * 512:kc * 512 + w],
                                start=True, stop=True)
                            if kc == nch - 1:
                                nc.vector.tensor_add(
                                    sap[:, w - P:w], sap[:, w - P:w],
                                    mask[:])
                            nc.vector.reduce_max(
                                mc[:, kc:kc + 1], sap[:], axis=AX)
                        m = small.tile([P, 1], f32, tag="m", name="m")
                        ball = balls[g]
                        if nch == 1:
                            nc.vector.tensor_scalar_add(
                                ball[:, bi:bi + 1], mc[:, 0:1], 0.125)
                        else:
                            nc.vector.reduce_max(m[:], mc[:, 0:nch],
                                                 axis=AX)
                            nc.vector.tensor_scalar_add(
                                ball[:, bi:bi + 1], m[:], 0.125)

                    def bchain(g):
                        # B columns -> row layout: [128,4] -> [4,128] ->
                        # [1,512]; bias lands in PSUM via -ones x brow
                        bt = btp.tile([4, P], f16, tag="bt", name="bt")
                        nc.tensor.transpose(bt[:], balls[g][:], identb[:])
                        bts = small.tile([4, P], f16, tag="bts", name="bts")
                        nc.scalar.copy(bts[:], bt[:])
                        brow = bfp.tile([1, 512], f16, tag="brow",
                                        name="brow")
                        nc.gpsimd.dma_start(
                            brow[0:1, :].rearrange("a (b c) -> a b c", b=4),
                            bts[:])
                        return brow

                    gbig, bigblocks = groups[0]
                    gsm, smblocks = groups[1]
                    for bi in range(4):
                        approx_block(gbig, bigblocks, bi)
                    brow = bchain(gbig)
                    weave = [(gsm, smblocks, bi) for bi in range(4)]

                    for gi, (g, blocks) in enumerate(groups):
                        oslot = g
                        nkt = blocks[-1] + 1
                        avg = avp.tile([HS + 1, 512], f32, tag="avg")
                        et_tiles = []
                        for kt in range(nkt):
                            if gi == 0 and weave and kt >= 2:
                                approx_block(*weave.pop(0))
                            sp = spool.tile([P, 512], f32, tag="S")
                            kb = KHL[:, kt * P:(kt + 1) * P]
                            nc.tensor.matmul(sp[:], lhsT=kb,
                                             rhs=qhh[:, g, :],
                                             start=True, stop=False)
                            nc.tensor.matmul(sp[:], lhsT=kb,
                                             rhs=qlz[:, g, :],
                                             start=False, stop=False)
                            nc.tensor.matmul(sp[:], lhsT=negones[:],
                                             rhs=brow[:],
                                             start=False, stop=True,
                                             skip_group_check=True)
                            if kt >= blocks[0]:
                                bi = kt - blocks[0]
                                nc.vector.tensor_add(
                                    sp[:, bi * P:(bi + 1) * P],
                                    sp[:, bi * P:(bi + 1) * P], maskT[:])
                            et = ets.tile([P, 512], f16, tag="et")
                            nc.scalar.activation(et[:], sp[:], EXP,
                                                 scale=SCALE)
                            et_tiles.append(et)
                            # AV for blocks whose causal span just completed,
                            # keeping accumulation groups sequential per tile
                            for bi, j in enumerate(blocks):
                                if j == kt:
                                    for ki in range(j + 1):
                                        nc.tensor.matmul(
                                            avg[:, bi * P:(bi + 1) * P],
                                            lhsT=vn2[:, ki, :],
                                            rhs=et_tiles[ki][
                                                :, bi * P:(bi + 1) * P],
                                            start=(ki == 0), stop=(ki == j),
                                            skip_group_check=True)
                        avs = osb.tile([HS + 1, 512], f32, tag="avs")
                        nc.vector.tensor_copy(avs[:], avg[:])
                        nc.sync.dma_start(
                            out_d[oslot * 65:(oslot + 1) * 65, :], avs[:])
                        if gi == 0:
                            while weave:
                                approx_block(*weave.pop(0))
                            brow = bchain(gsm)

                pid = nc.partition_id()
                with tc.If(pid < 4) as cmp:
                    emit_role(0)
                with cmp.Else():
                    emit_role(1)

    nc.compile()
    return nc


def _get_program():
    global _COMPILED
    if _COMPILED is None:
        _COMPILED = _build()
    return _COMPILED


def _install_ntff_hook():
    import sys, types
    if "antenv.axon_hooks" in sys.modules:
        return
    try:
        from trn_agent_boot.trn_boot import _ntff_profile_via_ctypes
        hook = _ntff_profile_via_ctypes("/opt/axon/libaxon_pjrt.so")
        mod = types.ModuleType("antenv.axon_hooks")
        mod.get_axon_ntff_profile_hook = lambda: hook
        mod.set_axon_ntff_profile_hook = lambda h: None
        import antenv
        sys.modules["antenv.axon_hooks"] = mod
        antenv.axon_hooks = mod
    except Exception:
        pass


def _split_pair16(a):
    hi = a.astype(np.float16)
    lo = (a - hi.astype(np.float32)).astype(np.float16)
    return hi, lo


def _host_prep(inputs):
    x = np.asarray(inputs["x"], dtype=np.float32)
    wq = np.asarray(inputs["Wq"], dtype=np.float32)
    wk = np.asarray(inputs["Wk"], dtype=np.float32)
    wv = np.asarray(inputs["Wv"], dtype=np.float32)

    xtf = np.ascontiguousarray(np.transpose(x, (0, 2, 1)))  # [B, D, T] f32
    xh, xl = _split_pair16(xtf)
    xt = np.stack([xh, xl], axis=2)                # [B, D, 2, T] fp16

    def _wprep(wt):
        # [D, M] -> [P, ND*M]: row p holds [dt, m] for d = dt*P + p
        m = wt.shape[1]
        return np.ascontiguousarray(
            wt.reshape(ND, P, m).transpose(1, 0, 2).reshape(P, ND * m))

    wkvT = np.concatenate([wv, wk], axis=0).T      # [D, 128]
    wkvh, wkvl = _split_pair16(_wprep(wkvT))
    wqT = wq.T                                     # [D, 64]
    wqh, wql = _split_pair16(_wprep(wqT))

    identb = np.eye(P, dtype=np.float16)
    r = np.arange(P)
    # mask[q, k]: masked (NEG) where k > q; maskT is its transpose
    mask = np.where(r[None, :] <= r[:, None], 0.0, NEG).astype(np.float32)
    maskT = np.ascontiguousarray(mask.T)

    shared = {"wkvh": wkvh, "wkvl": wkvl, "wqh": wqh, "wql": wql,
              "identb": identb, "mask": mask, "maskT": maskT}
    in_maps = []
    for c in range(N_CORES):
        m = dict(shared)
        m["xt"] = np.ascontiguousarray(xt[c % B])
        in_maps.append(m)
    return in_maps


def _run(inputs, trace=False):
    from concourse.bass_utils import run_bass_kernel_spmd

    if trace:
        _install_ntff_hook()
    nc = _get_program()
    in_maps = _host_prep(inputs)
    res = run_bass_kernel_spmd(nc, in_maps, list(range(N_CORES)), trace=trace)

    out = np.empty((B, T, HS), dtype=np.float32)
    for c in range(N_CORES):
        b, role = c % B, c // B
        oc = res.results[c]["out"]          # [130, 512]: 2 groups x [65, 512]
        for g in range(2):
            blocks = ROLE_BLOCKS[role][4 * g:4 * g + 4]
            avt = oc[65 * g:65 * g + 65]
            for bi, j in enumerate(blocks):
                sub = avt[0:HS, 128 * bi:128 * (bi + 1)]
                z = avt[HS:HS + 1, 128 * bi:128 * (bi + 1)]
                out[b, 128 * j:128 * (j + 1)] = (sub / z).T
    return out, res


def kernel(**inputs):
    out, _ = _run(inputs, trace=False)
    return out


# revision 63
# speedup vs baseline: 1.1622x; 1.1060x over previous
"""Single-head causal attention (B=4, T=2048, D=1024, HS=64) on 8 TRN2 cores.

Sharding: 2 cores per batch element. Query blocks (128 rows, 16/batch) are
fold-split for causal balance:
  role 0 (cores 0-3): blocks {0,1,2,3,12,13,14,15} of batch (core_id % 4)
  role 1 (cores 4-7): blocks {4..11}              of batch (core_id % 4)

Precision scheme (fp16, 10 mantissa bits):
  host: x.T in fp16 single plane (4MB/batch DMA), W pre-transposed fp16
  hi/lo pairs. k,v,q projections: 2 matmul groups (x*wh + x*wl) in fp32
  PSUM — error dominated by the single fp16 rounding of x (~2^-11 rel).
  scores: k,q re-split into fp16 hi/lo pairs on device;
    S = [qh;qh]^T.[kl;kh] + [0;ql]^T.[kl;kh]  (2 matmuls per 512-chunk)
  softmax: chunked row-max (DVE) + exp on ACT (scale=8, bias=-8*max),
  E fp16; E^T via PE transposes batched 4-per-PSUM-tile; AV inverted:
  out[q,h] accumulates lhsT=E^T-block (stationary, FWL) x rhs=v-natural
  so the result lands layout-natural and 1/Z applies per-partition.
"""

import numpy as np

N_CORES = 8
B, T, D, HS = 4, 2048, 1024, 64
P = 128
NT = T // P        # 16
ND = D // P        # 8
NCH = 4            # 512-wide t chunks
SCALE = 8.0        # sqrt(HS)
NEG = -1.0e30

ROLE_BLOCKS = [
    [0, 1, 2, 3, 12, 13, 14, 15],
    [4, 5, 6, 7, 8, 9, 10, 11],
]
ROLE_QCHUNKS = [[0, 3], [1, 2]]  # 512-chunk indices holding each role's q rows


def _block_qloc(role, j):
    if role == 0:
        return (0, 128 * j) if j < 4 else (1, 128 * (j - 12))
    return (0, 128 * (j - 4)) if j < 8 else (1, 128 * (j - 8))


_COMPILED = None


def _build():
    import concourse.bass as bass
    import concourse.tile as tile
    from concourse import bacc, mybir

    f32 = mybir.dt.float32
    f16 = mybir.dt.float16
    EXP = mybir.ActivationFunctionType.Exp
    AX = mybir.AxisListType.X

    nc = bacc.Bacc("TRN2", target_bir_lowering=False, debug=False,
                   num_devices=N_CORES)

    # x^T per batch, fp16 hi/lo planes interleaved per d-row: [D, 2, T]
    # (8KB contiguous DRAM rows -> max DMA packet efficiency)
    xt_d = nc.dram_tensor("xt", [D, 2, T], f16, kind="ExternalInput").ap()
    wkvh_d = nc.dram_tensor("wkvh", [P, ND * P], f16, kind="ExternalInput").ap()
    wkvl_d = nc.dram_tensor("wkvl", [P, ND * P], f16, kind="ExternalInput").ap()
    wqh_d = nc.dram_tensor("wqh", [P, ND * HS], f16, kind="ExternalInput").ap()
    wql_d = nc.dram_tensor("wql", [P, ND * HS], f16, kind="ExternalInput").ap()
    identb_d = nc.dram_tensor("identb", [P, P], f16, kind="ExternalInput").ap()
    mask_d = nc.dram_tensor("mask", [P, P], f32, kind="ExternalInput").ap()
    maskT_d = nc.dram_tensor("maskT", [P, P], f32, kind="ExternalInput").ap()
    # out^T per group: [65, 512] (64 head rows + Z row) x 2 groups
    out_d = nc.dram_tensor("out", [2 * 65, 512], f32, kind="ExternalOutput").ap()

    with tile.TileContext(nc) as tc:
        with tc.tile_pool(name="consts", bufs=1) as consts, \
             tc.tile_pool(name="big", bufs=1) as big:
            identb = consts.tile([P, P], f16)
            mask = consts.tile([P, P], f32)
            maskT = consts.tile([P, P], f32)
            wkvh = consts.tile([P, ND, P], f16)
            wkvl = consts.tile([P, ND, P], f16)
            wqh = consts.tile([P, ND, HS], f16)
            wql = consts.tile([P, ND, HS], f16)

            # kv weights first (gate the first matmul), then x tiles, then
            # the rest of the small consts
            nc.scalar.dma_start(wkvh[:], wkvh_d.rearrange("p (a h) -> p a h", a=ND))

            # x^T tiles: one per dt, both planes: [128, 2, 2048] fp16,
            # single 1MB DMA with 8KB rows; alternate the two HWDGE queues.
            xt = [big.tile([P, 2, T], f16, name=f"xt{dt}", tag=f"xt{dt}")
                  for dt in range(ND)]
            for dt in range(ND):
                eng = (nc.scalar, nc.sync)[dt % 2]
                eng.dma_start(xt[dt][:], xt_d[dt * P:(dt + 1) * P])
                if dt == 0:
                    nc.scalar.dma_start(
                        wkvl[:], wkvl_d.rearrange("p (a h) -> p a h", a=ND))
            nc.scalar.dma_start(wqh[:], wqh_d.rearrange("p (a h) -> p a h", a=ND))
            nc.scalar.dma_start(wql[:], wql_d.rearrange("p (a h) -> p a h", a=ND))
            nc.scalar.dma_start(identb[:], identb_d[:])
            nc.scalar.dma_start(mask[:], mask_d[:])
            nc.scalar.dma_start(maskT[:], maskT_d[:])

            # KHL: rows 0:64 = k_lo, rows 64:128 = k_hi
            KHL = big.tile([P, T], f16)
            vTb = big.tile([HS, T], f16)
            # v natural + a ones column (row Z accumulator trick)
            vn2 = big.tile([P, NT, HS + 1], f16)
            qhh = big.tile([P, 2, 512], f16)   # rows 0:64=q_hi, 64:128=q_hi
            qlz = big.tile([P, 2, 512], f16)   # rows 0:64=0,    64:128=q_lo
            negones = big.tile([1, P], f16)
            nc.vector.memset(qlz[0:HS, :, :], 0.0)
            nc.vector.memset(vn2[:, :, HS:HS + 1], 1.0)
            nc.vector.memset(negones[:], -1.0)

            # ---- k,v projections over full T (both roles) ----
            with tc.tile_pool(name="pps", bufs=2, space="PSUM") as pps, \
                 tc.tile_pool(name="kltmp", bufs=2) as klt:
                # HAM warm-up: dummy matmuls on the first-arrived weight tile
                # while the x DMA streams in, so real matmuls start at 2.4GHz
                wps = pps.tile([P, P], f32, tag="warm", bufs=1)
                for _ in range(40):
                    nc.tensor.matmul(wps[:], lhsT=wkvh[:, 0, :],
                                     rhs=wkvh[:, 0, :], start=True, stop=True)
                # dt-outer, chunk-inner: each stationary weight tile is
                # loaded once and streams all 4 chunks (4 open PSUM tiles
                # in separate banks), quartering the LDWEIGHTS serialization
                pss = [pps.tile([P, 512], f32, tag=f"proj{ch}",
                                name=f"ps{ch}", bufs=1) for ch in range(NCH)]
                for gi2, (w_t, pl) in enumerate(
                        ((wkvh, 0), (wkvh, 1), (wkvl, 0))):
                    for dt in range(ND):
                        for ch in range(NCH):
                            nc.tensor.matmul(
                                pss[ch][:], lhsT=w_t[:, dt, :],
                                rhs=xt[dt][:, pl,
                                           ch * 512:(ch + 1) * 512],
                                start=(gi2 == 0 and dt == 0),
                                stop=(gi2 == 2 and dt == ND - 1),
                                skip_group_check=True)
                for ch in range(NCH):
                    cs = slice(ch * 512, (ch + 1) * 512)
                    ps = pss[ch]
                    # rows 0:64 = v^T, rows 64:128 = k
                    nc.scalar.copy(vTb[:, cs], ps[0:HS, :])
                    nc.scalar.copy(KHL[HS:P, cs], ps[HS:P, :])
                    kl = klt.tile([P, 512], f16, tag="kl")
                    nc.vector.tensor_sub(kl[HS:P, :], ps[HS:P, :],
                                         KHL[HS:P, cs])
                    nc.gpsimd.dma_start(KHL[0:HS, cs], kl[HS:P, :])
                    # v^T -> v natural (fp16) for this chunk's 4 key-blocks
                    for tt in range(4 * ch, 4 * ch + 4):
                        vp = pps.tile([P, HS], f16, tag="vre")
                        nc.tensor.transpose(
                            vp[:], vTb[:, tt * P:(tt + 1) * P],
                            identb[0:HS, 0:HS])
                        nc.scalar.copy(vn2[:, tt, 0:HS], vp[:])

            # ---- role-specific: q projections + attention (S^T-direct) ----
            with tc.tile_pool(name="spool", bufs=5, space="PSUM") as spool, \
                 tc.tile_pool(name="avp", bufs=2, space="PSUM") as avp, \
                 tc.tile_pool(name="btp", bufs=1, space="PSUM") as btp, \
                 tc.tile_pool(name="ets", bufs=16) as ets, \
                 tc.tile_pool(name="bfp", bufs=2) as bfp, \
                 tc.tile_pool(name="small", bufs=4) as small, \
                 tc.tile_pool(name="osb", bufs=2) as osb, \
                 tc.tile_pool(name="qtmp", bufs=2) as qtp:

                def emit_role(role):
                    qps = [spool.tile([HS, 512], f32, tag="S",
                                      name=f"qps{qc}") for qc in range(2)]
                    for gi2, (w_t, pl) in enumerate(
                            ((wqh, 0), (wqh, 1), (wql, 0))):
                        for dt in range(ND):
                            for qc, ch in enumerate(ROLE_QCHUNKS[role]):
                                nc.tensor.matmul(
                                    qps[qc][:], lhsT=w_t[:, dt, :],
                                    rhs=xt[dt][:, pl,
                                               ch * 512:(ch + 1) * 512],
                                    start=(gi2 == 0 and dt == 0),
                                    stop=(gi2 == 2 and dt == ND - 1),
                                    skip_group_check=True)
                    for qc in range(2):
                        ps = qps[qc]
                        nc.scalar.copy(qhh[0:HS, qc, :], ps[:])
                        qt = qtp.tile([HS, 512], f16, tag="qt")
                        nc.vector.tensor_sub(qt[:], ps[:], qhh[0:HS, qc, :])
                        nc.gpsimd.dma_start(qhh[HS:P, qc, :], qhh[0:HS, qc, :])
                        nc.gpsimd.dma_start(qlz[HS:P, qc, :], qt[:])

                    # big group (blocks 4-7 of the role) runs FIRST: its
                    # diagonal mask-adds land only in the last 4 kts, so the
                    # small group's approx reduces weave into the mask-free
                    # stretch; small group last also shrinks the tail.
                    groups = [(1, ROLE_BLOCKS[role][4:8]),
                              (0, ROLE_BLOCKS[role][0:4])]
                    balls = {g: small.tile([P, 4], f16, tag=f"ball{g}",
                                           name=f"ball{g}")
                             for g, _ in groups}

                    def approx_block(g, blocks, bi):
                        j = blocks[bi]
                        L = 128 * (j + 1)
                        nch = (L + 511) // 512
                        off = bi * P
                        mc = small.tile([P, 4], f32, tag="mc", name="mc")
                        for kc in range(nch):
                            w = min(512, L - kc * 512)
                            sap = spool.tile([P, w], f32, tag="S",
                                             name="sap")
                            nc.tensor.matmul(
                                sap[:], lhsT=qhh[HS:P, g, off:off + P],
                                rhs=KHL[HS:P, kc * 512:kc * 512 + w],
                                start=True, stop=True)
                            if kc == nch - 1:
                                nc.vector.tensor_add(
                                    sap[:, w - P:w], sap[:, w - P:w],
                                    mask[:])
                            nc.vector.reduce_max(
                                mc[:, kc:kc + 1], sap[:], axis=AX)
                        m = small.tile([P, 1], f32, tag="m", name="m")
                        ball = balls[g]
                        if nch == 1:
                            nc.vector.tensor_scalar_add(
                                ball[:, bi:bi + 1], mc[:, 0:1], 0.125)
                        else:
                            nc.vector.reduce_max(m[:], mc[:, 0:nch],
                                                 axis=AX)
                            nc.vector.tensor_scalar_add(
                                ball[:, bi:bi + 1], m[:], 0.125)

                    def bchain(g):
                        # B columns -> row layout: [128,4] -> [4,128] ->
                        # [1,512]; bias lands in PSUM via -ones x brow
                        bt = btp.tile([4, P], f16, tag="bt", name="bt")
                        nc.tensor.transpose(bt[:], balls[g][:], identb[:])
                        bts = small.tile([4, P], f16, tag="bts", name="bts")
                        nc.scalar.copy(bts[:], bt[:])
                        brow = bfp.tile([1, 512], f16, tag="brow",
                                        name="brow")
                        nc.gpsimd.dma_start(
                            brow[0:1, :].rearrange("a (b c) -> a b c", b=4),
                            bts[:])
                        return brow

                    gbig, bigblocks = groups[0]
                    gsm, smblocks = groups[1]
                    for bi in range(4):
                        approx_block(gbig, bigblocks, bi)
                    brow = bchain(gbig)
                    weave = [(gsm, smblocks, bi) for bi in range(4)]

                    DELAY = 2
                    for gi, (g, blocks) in enumerate(groups):
                        oslot = g
                        nkt = blocks[-1] + 1
                        avg = avp.tile([HS + 1, 512], f32, tag="avg")
                        et_tiles = []
                        sps = {}
                        # data MMs run DELAY kts ahead of the brow-dependent
                        # bias MM, so the PE queue isn't stalled on the
                        # approx-reduce chain
                        for step in range(nkt + DELAY):
                            if step < nkt:
                                kt = step
                                if gi == 0 and weave and kt >= 2:
                                    approx_block(*weave.pop(0))
                                sp = spool.tile([P, 512], f32, tag="S",
                                                name="sp")
                                kb = KHL[:, kt * P:(kt + 1) * P]
                                nc.tensor.matmul(sp[:], lhsT=kb,
                                                 rhs=qhh[:, g, :],
                                                 start=True, stop=False)
                                nc.tensor.matmul(sp[:], lhsT=kb,
                                                 rhs=qlz[:, g, :],
                                                 start=False, stop=True)
                                sps[kt] = sp
                            kb2 = step - DELAY
                            if kb2 < 0:
                                continue
                            kt = kb2
                            sp = sps.pop(kt)
                            nc.tensor.matmul(sp[:], lhsT=negones[:],
                                             rhs=brow[:],
                                             start=False, stop=True,
                                             skip_group_check=True)
                            if kt >= blocks[0]:
                                bi = kt - blocks[0]
                                nc.vector.tensor_add(
                                    sp[:, bi * P:(bi + 1) * P],
                                    sp[:, bi * P:(bi + 1) * P], maskT[:])
                            et = ets.tile([P, 512], f16, tag="et")
                            nc.scalar.activation(et[:], sp[:], EXP,
                                                 scale=SCALE)
                            et_tiles.append(et)
                            # AV for blocks whose causal span just completed,
                            # keeping accumulation groups sequential per tile
                            for bi, j in enumerate(blocks):
                                if j == kt:
                                    for ki in range(j + 1):
                                        nc.tensor.matmul(
                                            avg[:, bi * P:(bi + 1) * P],
                                            lhsT=vn2[:, ki, :],
                                            rhs=et_tiles[ki][
                                                :, bi * P:(bi + 1) * P],
                                            start=(ki == 0), stop=(ki == j),
                                            skip_group_check=True)
                        avs = osb.tile([HS + 1, 512], f32, tag="avs")
                        nc.vector.tensor_copy(avs[:], avg[:])
                        nc.sync.dma_start(
                            out_d[oslot * 65:(oslot + 1) * 65, :], avs[:])
                        if gi == 0:
                            while weave:
                                approx_block(*weave.pop(0))
                            brow = bchain(gsm)

                pid = nc.partition_id()
                with tc.If(pid < 4) as cmp:
                    emit_role(0)
                with cmp.Else():
                    emit_role(1)

    nc.compile()
    return nc


def _get_program():
    global _COMPILED
    if _COMPILED is None:
        _COMPILED = _build()
    return _COMPILED


def _install_ntff_hook():
    import sys, types
    if "antenv.axon_hooks" in sys.modules:
        return
    try:
        from trn_agent_boot.trn_boot import _ntff_profile_via_ctypes
        hook = _ntff_profile_via_ctypes("/opt/axon/libaxon_pjrt.so")
        mod = types.ModuleType("antenv.axon_hooks")
        mod.get_axon_ntff_profile_hook = lambda: hook
        mod.set_axon_ntff_profile_hook = lambda h: None
        import antenv
        sys.modules["antenv.axon_hooks"] = mod
        antenv.axon_hooks = mod
    except Exception:
        pass


def _split_pair16(a):
    hi = a.astype(np.float16)
    lo = (a - hi.astype(np.float32)).astype(np.float16)
    return hi, lo


def _host_prep(inputs):
    x = np.asarray(inputs["x"], dtype=np.float32)
    wq = np.asarray(inputs["Wq"], dtype=np.float32)
    wk = np.asarray(inputs["Wk"], dtype=np.float32)
    wv = np.asarray(inputs["Wv"], dtype=np.float32)

    xtf = np.ascontiguousarray(np.transpose(x, (0, 2, 1)))  # [B, D, T] f32
    xh, xl = _split_pair16(xtf)
    xt = np.stack([xh, xl], axis=2)                # [B, D, 2, T] fp16

    def _wprep(wt):
        # [D, M] -> [P, ND*M]: row p holds [dt, m] for d = dt*P + p
        m = wt.shape[1]
        return np.ascontiguousarray(
            wt.reshape(ND, P, m).transpose(1, 0, 2).reshape(P, ND * m))

    wkvT = np.concatenate([wv, wk], axis=0).T      # [D, 128]
    wkvh, wkvl = _split_pair16(_wprep(wkvT))
    wqT = wq.T                                     # [D, 64]
    wqh, wql = _split_pair16(_wprep(wqT))

    identb = np.eye(P, dtype=np.float16)
    r = np.arange(P)
    # mask[q, k]: masked (NEG) where k > q; maskT is its transpose
    mask = np.where(r[None, :] <= r[:, None], 0.0, NEG).astype(np.float32)
    maskT = np.ascontiguousarray(mask.T)

    shared = {"wkvh": wkvh, "wkvl": wkvl, "wqh": wqh, "wql": wql,
              "identb": identb, "mask": mask, "maskT": maskT}
    in_maps = []
    for c in range(N_CORES):
        m = dict(shared)
        m["xt"] = np.ascontiguousarray(xt[c % B])
        in_maps.append(m)
    return in_maps


def _run(inputs, trace=False):
    from concourse.bass_utils import run_bass_kernel_spmd

    if trace:
        _install_ntff_hook()
    nc = _get_program()
    in_maps = _host_prep(inputs)
    res = run_bass_kernel_spmd(nc, in_maps, list(range(N_CORES)), trace=trace)

    out = np.empty((B, T, HS), dtype=np.float32)
    for c in range(N_CORES):
        b, role = c % B, c // B
        oc = res.results[c]["out"]          # [130, 512]: 2 groups x [65, 512]
        for g in range(2):
            blocks = ROLE_BLOCKS[role][4 * g:4 * g + 4]
            avt = oc[65 * g:65 * g + 65]
            for bi, j in enumerate(blocks):
                sub = avt[0:HS, 128 * bi:128 * (bi + 1)]
                z = avt[HS:HS + 1, 128 * bi:128 * (bi + 1)]
                out[b, 128 * j:128 * (j + 1)] = (sub / z).T
    return out, res


def kernel(**inputs):
    out, _ = _run(inputs, trace=False)
    return out
